# revision 12
# baseline (speedup 1.0000x reference)
"""Trainium2 Bass kernel for AtomActionPredictor: gather + 2-layer MLP.

Strategy (data parallel over 8 NeuronCores):
  - The 400k rc_indices are deduplicated host-side (np.unique); the ~330k
    unique atom rows are gathered host-side during sharding and dealt in
    equal contiguous chunks to the 8 cores, shipped bf16 and pre-transposed
    to the matmul-rhs orientation [128, 2, tpad] ([p, c, t] = row_t[c*128+p]).
  - Each core streams its chunk through a 2-layer MLP (bf16 matmuls with
    f32 PSUM accumulation, exact-erf GELU + bias on the scalar engine,
    second bias on the vector engine) and writes logits [VOCAB, tpad] bf16.
  - The host expands duplicates / restores token order with one vectorized
    take, and casts to f32.
  This keeps every engine off the critical path except DMA: per core
  ~21 MB in + ~11 MB out of HBM traffic, so the kernel sits at the
  memory roofline instead of the GPSIMD descriptor-generation limit that
  bounds an on-device dma_gather.
"""
import numpy as np
import ml_dtypes

import concourse.bass as bass
import concourse.mybir as mybir
import concourse.tile as tile
from concourse import bacc
from concourse.bass_utils import run_bass_kernel_spmd

N_CORES = 8
ATOM_DIM = 256
HIDDEN = 128
VOCAB = 128

N_CHUNK = 512        # matmul moving-dim chunk (one PSUM bank)
BIG = 8192           # tokens per input DMA macro-chunk (4 MB in)

F32 = mybir.dt.float32
BF16 = mybir.dt.bfloat16


def _round_up(x, m):
    return (x + m - 1) // m * m


def build_graph(tpad):
    """Streaming MLP over tpad pre-gathered tokens (tpad % N_CHUNK == 0).

    Input DMAs (2 MB blocks) go on the sync HWDGE queue; weight loads and
    output writes go on the scalar HWDGE queue so the first input transfer
    leads its FIFO and output writes drain per OUT_SUB tokens (short tail).
    """
    nc = bacc.Bacc("TRN2", target_bir_lowering=False, debug=False,
                   num_devices=N_CORES)
    xin = nc.dram_tensor("xin", [128, 2, tpad], BF16, kind="ExternalInput")
    w1p = nc.dram_tensor("w1p", [128, 2 * HIDDEN], BF16, kind="ExternalInput")
    w2p = nc.dram_tensor("w2p", [HIDDEN, VOCAB], BF16, kind="ExternalInput")
    b1p = nc.dram_tensor("b1p", [HIDDEN, 1], F32, kind="ExternalInput")
    b2p = nc.dram_tensor("b2p", [VOCAB, 1], F32, kind="ExternalInput")
    out = nc.dram_tensor("out", [VOCAB, tpad], BF16, kind="ExternalOutput")

    act_fn = mybir.ActivationFunctionType.Gelu

    # block sizes: full BIG blocks, then a 1024-token taper at the end so
    # the pipeline drain after the last input transfer is short
    blocks = []
    t = 0
    while tpad - t > BIG + 2048:
        blocks.append((t, BIG))
        t += BIG
    while t < tpad:
        bn = min(1024, tpad - t)
        blocks.append((t, bn))
        t += bn

    with tile.TileContext(nc) as tc:
        with (
            tc.tile_pool(name="const", bufs=1) as cpool,
            tc.tile_pool(name="xt", bufs=3) as xt_pool,
            tc.tile_pool(name="ht", bufs=4) as ht_pool,
            tc.tile_pool(name="osb", bufs=3) as osb_pool,
            tc.tile_pool(name="psh", bufs=4, space="PSUM") as psh_pool,
            tc.tile_pool(name="pso", bufs=4, space="PSUM") as pso_pool,
        ):
            # first input block leads the sync queue FIFO
            xt0 = xt_pool.tile([128, 2, blocks[0][1]], BF16, tag="xt")
            nc.sync.dma_start(out=xt0[:], in_=xin[:, :, 0:blocks[0][1]])

            w1_sb = cpool.tile([128, 2 * HIDDEN], BF16)
            nc.scalar.dma_start(out=w1_sb[:], in_=w1p[:])
            w2_sb = cpool.tile([HIDDEN, VOCAB], BF16)
            nc.scalar.dma_start(out=w2_sb[:], in_=w2p[:])
            b1_sb = cpool.tile([HIDDEN, 1], F32)
            nc.scalar.dma_start(out=b1_sb[:], in_=b1p[:])
            b2_sb = cpool.tile([VOCAB, 1], F32)
            nc.scalar.dma_start(out=b2_sb[:], in_=b2p[:])

            for bi, (t0, bn) in enumerate(blocks):
                if bi == 0:
                    xt = xt0
                else:
                    xt = xt_pool.tile([128, 2, bn], BF16, tag="xt")
                    nc.sync.dma_start(out=xt[:], in_=xin[:, :, t0:t0 + bn])
                osb = osb_pool.tile([VOCAB, bn], BF16, tag="osb")
                for n0 in range(0, bn, N_CHUNK):
                    n = min(N_CHUNK, bn - n0)
                    ht = ht_pool.tile([HIDDEN, n], BF16, tag="ht")
                    ps_h = psh_pool.tile([HIDDEN, n], F32, tag="psh")
                    nc.tensor.matmul(ps_h[:], lhsT=w1_sb[:, 0:HIDDEN],
                                     rhs=xt[:, 0, n0:n0 + n],
                                     start=True, stop=False)
                    nc.tensor.matmul(ps_h[:],
                                     lhsT=w1_sb[:, HIDDEN:2 * HIDDEN],
                                     rhs=xt[:, 1, n0:n0 + n],
                                     start=False, stop=True)
                    nc.scalar.activation(ht[:], ps_h[:], act_fn,
                                         bias=b1_sb[:, 0:1], scale=1.0)
                    ps_o = pso_pool.tile([VOCAB, n], F32, tag="pso")
                    nc.tensor.matmul(ps_o[:], lhsT=w2_sb[:], rhs=ht[:],
                                     start=True, stop=True)
                    nc.vector.tensor_tensor(
                        out=osb[:, n0:n0 + n], in0=ps_o[:],
                        in1=b2_sb[:, 0:1].to_broadcast([VOCAB, n]),
                        op=mybir.AluOpType.add)
                nc.gpsimd.dma_start(out=out[:, t0:t0 + bn], in_=osb[:])
    nc.compile()
    return nc


def kernel(atom_features, rc_indices, W1, b1, W2, b2):
    atom_features = np.asarray(atom_features)
    rc_indices = np.asarray(rc_indices)
    n_rc = rc_indices.shape[0]

    # Host-side shard prep: dedupe indices, gather unique rows, deal equal
    # contiguous chunks to the cores in matmul-rhs orientation.
    uniq, inv = np.unique(rc_indices, return_inverse=True)
    n_uniq = uniq.shape[0]
    per_core = -(-n_uniq // N_CORES)
    tpad = max(N_CHUNK, _round_up(per_core, N_CHUNK))

    rows = atom_features[uniq].astype(ml_dtypes.bfloat16)  # [U, 256]

    xins = []
    for c in range(N_CORES):
        x = rows[c * per_core:(c + 1) * per_core]
        if x.shape[0] < tpad:
            x = np.concatenate(
                [x, np.zeros((tpad - x.shape[0], ATOM_DIM), x.dtype)])
        # [t, 256] -> [p, c, t] with [p, c, t] = x[t, c*128 + p]
        xins.append(np.ascontiguousarray(
            x.reshape(tpad, 2, 128).transpose(2, 1, 0)))

    w1p = np.ascontiguousarray(
        np.asarray(W1).reshape(2, 128, HIDDEN).transpose(1, 0, 2)
        .reshape(128, 2 * HIDDEN)).astype(ml_dtypes.bfloat16)
    w2p = np.asarray(W2).astype(ml_dtypes.bfloat16)
    b1p = np.asarray(b1).reshape(HIDDEN, 1).astype(np.float32)
    b2p = np.asarray(b2).reshape(VOCAB, 1).astype(np.float32)

    nc = build_graph(tpad)

    in_maps = [{"xin": xins[c], "w1p": w1p, "w2p": w2p,
                "b1p": b1p, "b2p": b2p} for c in range(N_CORES)]
    res = run_bass_kernel_spmd(nc, in_maps, core_ids=list(range(N_CORES)))

    # [VOCAB, N_CORES * tpad] -> expand duplicates & restore token order
    full = np.concatenate([res.results[c]["out"] for c in range(N_CORES)],
                          axis=1)
    slot = (inv // per_core) * tpad + (inv % per_core)
    logits = full.T[slot].astype(np.float32)
    assert logits.shape == (n_rc, VOCAB)
    return logits


# revision 13
# speedup vs baseline: 1.2285x; 1.2285x over previous
"""Trainium2 Bass kernel for AtomActionPredictor: gather + 2-layer MLP.

Strategy (data parallel over 8 NeuronCores):
  - The 400k rc_indices are deduplicated host-side (np.unique); the ~330k
    unique atom rows are gathered host-side during sharding and dealt in
    equal contiguous chunks to the 8 cores, shipped bf16 and pre-transposed
    to the matmul-rhs orientation [128, 2, tpad] ([p, c, t] = row_t[c*128+p]).
  - Each core streams its chunk through a 2-layer MLP (bf16 matmuls with
    f32 PSUM accumulation, exact-erf GELU + bias on the scalar engine,
    second bias on the vector engine) and writes logits [VOCAB, tpad] bf16.
  - The host expands duplicates / restores token order with one vectorized
    take, and casts to f32.
  This keeps every engine off the critical path except DMA: per core
  ~21 MB in + ~11 MB out of HBM traffic, so the kernel sits at the
  memory roofline instead of the GPSIMD descriptor-generation limit that
  bounds an on-device dma_gather.
"""
import numpy as np
import ml_dtypes

import concourse.bass as bass
import concourse.mybir as mybir
import concourse.tile as tile
from concourse import bacc
from concourse.bass_utils import run_bass_kernel_spmd

N_CORES = 8
ATOM_DIM = 256
HIDDEN = 128
VOCAB = 128

N_CHUNK = 512        # matmul moving-dim chunk (one PSUM bank)
BIG = 4096           # tokens per input DMA macro-chunk (2 MB in)

F32 = mybir.dt.float32
BF16 = mybir.dt.bfloat16


def _round_up(x, m):
    return (x + m - 1) // m * m


def build_graph(tpad):
    """Streaming MLP over tpad pre-gathered tokens (tpad % N_CHUNK == 0).

    Input DMAs (2 MB blocks) go on the sync HWDGE queue; weight loads and
    output writes go on the scalar HWDGE queue so the first input transfer
    leads its FIFO and output writes drain per OUT_SUB tokens (short tail).
    """
    nc = bacc.Bacc("TRN2", target_bir_lowering=False, debug=False,
                   num_devices=N_CORES)
    xin = nc.dram_tensor("xin", [128, 2, tpad], BF16, kind="ExternalInput")
    w1p = nc.dram_tensor("w1p", [128, 2 * HIDDEN], BF16, kind="ExternalInput")
    w2p = nc.dram_tensor("w2p", [HIDDEN, VOCAB], BF16, kind="ExternalInput")
    b1p = nc.dram_tensor("b1p", [HIDDEN, 1], F32, kind="ExternalInput")
    b2p = nc.dram_tensor("b2p", [VOCAB, 1], F32, kind="ExternalInput")
    out = nc.dram_tensor("out", [VOCAB, tpad], BF16, kind="ExternalOutput")

    act_fn = mybir.ActivationFunctionType.Gelu

    # block sizes: full BIG blocks, then a 1024-token taper at the end so
    # the pipeline drain after the last input transfer is short
    blocks = []
    t = 0
    while tpad - t > BIG + 2048:
        blocks.append((t, BIG))
        t += BIG
    while t < tpad:
        bn = min(1024, tpad - t)
        blocks.append((t, bn))
        t += bn

    with tile.TileContext(nc) as tc:
        with (
            tc.tile_pool(name="const", bufs=1) as cpool,
            tc.tile_pool(name="xt", bufs=4) as xt_pool,
            tc.tile_pool(name="ht", bufs=4) as ht_pool,
            tc.tile_pool(name="osb", bufs=6) as osb_pool,
            tc.tile_pool(name="psh", bufs=4, space="PSUM") as psh_pool,
            tc.tile_pool(name="pso", bufs=4, space="PSUM") as pso_pool,
        ):
            # first input block leads the sync queue FIFO
            xt0 = xt_pool.tile([128, 2, blocks[0][1]], BF16, tag="xt")
            nc.sync.dma_start(out=xt0[:], in_=xin[:, :, 0:blocks[0][1]])

            w1_sb = cpool.tile([128, 2 * HIDDEN], BF16)
            nc.scalar.dma_start(out=w1_sb[:], in_=w1p[:])
            w2_sb = cpool.tile([HIDDEN, VOCAB], BF16)
            nc.scalar.dma_start(out=w2_sb[:], in_=w2p[:])
            b1_sb = cpool.tile([HIDDEN, 1], F32)
            nc.scalar.dma_start(out=b1_sb[:], in_=b1p[:])
            b2_sb = cpool.tile([VOCAB, 1], F32)
            nc.scalar.dma_start(out=b2_sb[:], in_=b2p[:])

            for bi, (t0, bn) in enumerate(blocks):
                if bi == 0:
                    xt = xt0
                else:
                    xt = xt_pool.tile([128, 2, bn], BF16, tag="xt")
                    nc.sync.dma_start(out=xt[:], in_=xin[:, :, t0:t0 + bn])
                osb = osb_pool.tile([VOCAB, bn], BF16, tag="osb")
                for n0 in range(0, bn, N_CHUNK):
                    n = min(N_CHUNK, bn - n0)
                    ht = ht_pool.tile([HIDDEN, n], BF16, tag="ht")
                    ps_h = psh_pool.tile([HIDDEN, n], F32, tag="psh")
                    nc.tensor.matmul(ps_h[:], lhsT=w1_sb[:, 0:HIDDEN],
                                     rhs=xt[:, 0, n0:n0 + n],
                                     start=True, stop=False)
                    nc.tensor.matmul(ps_h[:],
                                     lhsT=w1_sb[:, HIDDEN:2 * HIDDEN],
                                     rhs=xt[:, 1, n0:n0 + n],
                                     start=False, stop=True)
                    nc.scalar.activation(ht[:], ps_h[:], act_fn,
                                         bias=b1_sb[:, 0:1], scale=1.0)
                    ps_o = pso_pool.tile([VOCAB, n], F32, tag="pso")
                    nc.tensor.matmul(ps_o[:], lhsT=w2_sb[:], rhs=ht[:],
                                     start=True, stop=True)
                    nc.vector.tensor_tensor(
                        out=osb[:, n0:n0 + n], in0=ps_o[:],
                        in1=b2_sb[:, 0:1].to_broadcast([VOCAB, n]),
                        op=mybir.AluOpType.add)
                if bi == len(blocks) - 1:
                    nc.scalar.dma_start(out=out[:, t0:t0 + bn], in_=osb[:])
                else:
                    nc.gpsimd.dma_start(out=out[:, t0:t0 + bn], in_=osb[:])
    nc.compile()
    return nc


def kernel(atom_features, rc_indices, W1, b1, W2, b2):
    atom_features = np.asarray(atom_features)
    rc_indices = np.asarray(rc_indices)
    n_rc = rc_indices.shape[0]

    # Host-side shard prep: dedupe indices, gather unique rows, deal equal
    # contiguous chunks to the cores in matmul-rhs orientation.
    uniq, inv = np.unique(rc_indices, return_inverse=True)
    n_uniq = uniq.shape[0]
    per_core = -(-n_uniq // N_CORES)
    tpad = max(N_CHUNK, _round_up(per_core, 128))

    rows = atom_features[uniq].astype(ml_dtypes.bfloat16)  # [U, 256]

    xins = []
    for c in range(N_CORES):
        x = rows[c * per_core:(c + 1) * per_core]
        if x.shape[0] < tpad:
            x = np.concatenate(
                [x, np.zeros((tpad - x.shape[0], ATOM_DIM), x.dtype)])
        # [t, 256] -> [p, c, t] with [p, c, t] = x[t, c*128 + p]
        xins.append(np.ascontiguousarray(
            x.reshape(tpad, 2, 128).transpose(2, 1, 0)))

    w1p = np.ascontiguousarray(
        np.asarray(W1).reshape(2, 128, HIDDEN).transpose(1, 0, 2)
        .reshape(128, 2 * HIDDEN)).astype(ml_dtypes.bfloat16)
    w2p = np.asarray(W2).astype(ml_dtypes.bfloat16)
    b1p = np.asarray(b1).reshape(HIDDEN, 1).astype(np.float32)
    b2p = np.asarray(b2).reshape(VOCAB, 1).astype(np.float32)

    nc = build_graph(tpad)

    in_maps = [{"xin": xins[c], "w1p": w1p, "w2p": w2p,
                "b1p": b1p, "b2p": b2p} for c in range(N_CORES)]
    res = run_bass_kernel_spmd(nc, in_maps, core_ids=list(range(N_CORES)))

    # [VOCAB, N_CORES * tpad] -> expand duplicates & restore token order
    full = np.concatenate([res.results[c]["out"] for c in range(N_CORES)],
                          axis=1)
    slot = (inv // per_core) * tpad + (inv % per_core)
    logits = full.T[slot].astype(np.float32)
    assert logits.shape == (n_rc, VOCAB)
    return logits
